# revision 1
# baseline (speedup 1.0000x reference)
"""Trainium2 Bass kernel for nn_BestNet_46196668236142 (LRU block).

Pipeline per token: LN1 -> leaky -> complex diagonal recurrence over T
-> y = Re(C h) + D z -> leaky(LN2) -> MLP -> LN3 -> +skip.

Strategy:
- Data-parallel: shard B=32 across 8 cores (4 samples/core).
- The complex recurrence h_t = lam*h_{t-1} + u_t (lam = r*e^{i th}) is
  decoupled into two REAL per-channel scans via polar rotation:
      g_t = e^{-i th t} h_t   =>   g_t = r * g_{t-1} + e^{-i th t} u_t
  which maps onto the HW tensor_tensor_scan (op0=mult, op1=add) along
  the free (time) axis, n on partitions. Pre/post rotations use
  host-precomputed cos/sin tables; the post-rotation adds are folded
  into the C-projection matmuls (4 rotated streams A1..A4).
- Chunked over time (CT=512) with a tiny [P,1] carry between chunks.
"""

import os
import sys

import numpy as np

for _p in ("/opt/trn_rl_repo", "/root/.axon_site/_ro/trn_rl_repo"):
    if os.path.isdir(_p) and _p not in sys.path:
        sys.path.insert(0, _p)

import concourse.bass as bass
import concourse.mybir as mybir
from concourse import bacc, masks, tile
from concourse.bass_utils import run_bass_kernel_spmd

B, T, D, N = 32, 4096, 256, 256
NCORES = 8
BS = B // NCORES            # batches per core
CT = 512                    # time chunk
NSUB = CT // 128            # token subtiles per chunk
NCH = T // CT               # chunks per batch
EPS = 1e-5
SLOPE = 0.01
F32 = mybir.dt.float32
AO = mybir.AluOpType
AF = mybir.ActivationFunctionType

_PROG_CACHE = {}


def _build_program(flags):
    """flags = (g1, g2, g3, bias, mask) booleans for the general path."""
    g1, g2, g3, use_bias, use_mask = flags
    nc = bacc.Bacc(None, target_bir_lowering=False)

    x_d = nc.declare_dram_parameter("x", [BS, T, D], F32, isOutput=False)
    q0r_d = nc.declare_dram_parameter("q0r", [BS, N], F32, isOutput=False)
    q0i_d = nc.declare_dram_parameter("q0i", [BS, N], F32, isOutput=False)
    cos_d = nc.declare_dram_parameter("cosj", [N, CT], F32, isOutput=False)
    sin_d = nc.declare_dram_parameter("sinj", [N, CT], F32, isOutput=False)
    cneg_d = nc.declare_dram_parameter("cneg", [N, CT], F32, isOutput=False)
    sneg_d = nc.declare_dram_parameter("sneg", [N, CT], F32, isOutput=False)
    rbc_d = nc.declare_dram_parameter("rbc", [N, CT], F32, isOutput=False)
    ecl_d = nc.declare_dram_parameter("ecl", [N, 1], F32, isOutput=False)
    esl_d = nc.declare_dram_parameter("esl", [N, 1], F32, isOutput=False)
    brt_d = nc.declare_dram_parameter("BrT", [D, N], F32, isOutput=False)
    bit_d = nc.declare_dram_parameter("BiT", [D, N], F32, isOutput=False)
    crt_d = nc.declare_dram_parameter("CrT", [N, D], F32, isOutput=False)
    cit_d = nc.declare_dram_parameter("CiT", [N, D], F32, isOutput=False)
    dt_d = nc.declare_dram_parameter("DT", [D, N], F32, isOutput=False)
    mt_d = nc.declare_dram_parameter("MT", [N, D], F32, isOutput=False)
    out_d = nc.declare_dram_parameter("out", [BS, T, D], F32, isOutput=True)

    if use_mask:
        d0_d = nc.declare_dram_parameter("d0tab", [BS, N, T], F32, isOutput=False)
    gb_params = {}
    for name, on in (("g1", g1), ("b1", g1), ("g2", g2), ("b2", g2),
                     ("g3", g3), ("b3", g3), ("mb", use_bias)):
        if on:
            gb_params[name] = nc.declare_dram_parameter(name + "bc", [128, D], F32)

    from contextlib import ExitStack

    with tile.TileContext(nc) as tc, ExitStack() as ctx:
        cpool = ctx.enter_context(tc.tile_pool(name="consts", bufs=1))

        _cn = [0]

        def cload(dram, shape):
            _cn[0] += 1
            t = cpool.tile(shape, F32, name=f"const{_cn[0]}",
                           tag=f"const{_cn[0]}")
            nc.sync.dma_start(t[:], dram)
            return t

        # constants: tables split by n-chunk
        cosj = [cload(cos_d[p * 128:(p + 1) * 128, :], [128, CT]) for p in range(2)]
        sinj = [cload(sin_d[p * 128:(p + 1) * 128, :], [128, CT]) for p in range(2)]
        cneg = [cload(cneg_d[p * 128:(p + 1) * 128, :], [128, CT]) for p in range(2)]
        sneg = [cload(sneg_d[p * 128:(p + 1) * 128, :], [128, CT]) for p in range(2)]
        rbc = [cload(rbc_d[p * 128:(p + 1) * 128, :], [128, CT]) for p in range(2)]
        ecl = [cload(ecl_d[p * 128:(p + 1) * 128, :], [128, 1]) for p in range(2)]
        esl = [cload(esl_d[p * 128:(p + 1) * 128, :], [128, 1]) for p in range(2)]
        brt = [cload(brt_d[k * 128:(k + 1) * 128, :], [128, N]) for k in range(2)]
        bit = [cload(bit_d[k * 128:(k + 1) * 128, :], [128, N]) for k in range(2)]
        crt = [cload(crt_d[p * 128:(p + 1) * 128, :], [128, D]) for p in range(2)]
        cit = [cload(cit_d[p * 128:(p + 1) * 128, :], [128, D]) for p in range(2)]
        dts = [cload(dt_d[k * 128:(k + 1) * 128, :], [128, N]) for k in range(2)]
        mts = [cload(mt_d[p * 128:(p + 1) * 128, :], [128, D]) for p in range(2)]
        gbt = {k: cload(v[:, :], [128, D]) for k, v in gb_params.items()}
        ident = cpool.tile([128, 128], F32)
        masks.make_identity(nc, ident[:])
        epst = cpool.tile([128, 1], F32)
        nc.gpsimd.memset(epst[:], EPS)

        xin = ctx.enter_context(tc.tile_pool(name="xin", bufs=6))
        statp = ctx.enter_context(tc.tile_pool(name="stat", bufs=24))
        zskip = ctx.enter_context(tc.tile_pool(name="zskip", bufs=2 * NSUB + 2))
        zlp = ctx.enter_context(tc.tile_pool(name="zl", bufs=4))
        ztp = ctx.enter_context(tc.tile_pool(name="zt", bufs=4))
        ptr = ctx.enter_context(
            tc.tile_pool(name="ptr", bufs=2, space=bass.MemorySpace.PSUM))
        pu = ctx.enter_context(
            tc.tile_pool(name="pu", bufs=4, space=bass.MemorySpace.PSUM))
        tmp = ctx.enter_context(tc.tile_pool(name="tmp", bufs=6))
        wp = ctx.enter_context(tc.tile_pool(name="w", bufs=4))
        gp = ctx.enter_context(tc.tile_pool(name="g", bufs=6))
        gip = ctx.enter_context(tc.tile_pool(name="gi", bufs=10))
        ap_ = ctx.enter_context(tc.tile_pool(name="astr", bufs=10))
        py1 = ctx.enter_context(
            tc.tile_pool(name="py1", bufs=1, space=bass.MemorySpace.PSUM))
        yl2p = ctx.enter_context(tc.tile_pool(name="yl2", bufs=4))
        y2tp = ctx.enter_context(tc.tile_pool(name="y2t", bufs=4))
        py3 = ctx.enter_context(
            tc.tile_pool(name="py3", bufs=1, space=bass.MemorySpace.PSUM))
        yop = ctx.enter_context(tc.tile_pool(name="yo", bufs=6))
        d0p = ctx.enter_context(tc.tile_pool(name="d0p", bufs=4))

        def ln_scale_bias(src_ap):
            """Return (rstd, negmu_rstd) [128,1] tiles for a [128, D] input."""
            st6 = statp.tile([128, 6], F32)
            nc.vector.bn_stats(st6[:], src_ap)
            mv = statp.tile([128, 2], F32)
            nc.vector.bn_aggr(mv[:], st6[:])
            std = statp.tile([128, 1], F32)
            nc.scalar.activation(std[:], mv[:, 1:2], AF.Sqrt, bias=epst[:])
            rstd = statp.tile([128, 1], F32)
            nc.vector.reciprocal(rstd[:], std[:])
            nmr = statp.tile([128, 1], F32)
            nc.vector.scalar_tensor_tensor(
                nmr[:], mv[:, 0:1], -1.0, rstd[:], op0=AO.mult, op1=AO.mult)
            return rstd, nmr

        for b in range(BS):
            ginit = {}
            for p in range(2):
                for comp, src in ((0, q0r_d), (1, q0i_d)):
                    t = gip.tile([128, 1], F32, name="giq")
                    nc.sync.dma_start(t[:], src[b, p * 128:(p + 1) * 128])
                    ginit[(p, comp)] = t
            for c in range(NCH):
                t0 = c * CT
                # ---- stage 1: load, LN1, leaky, transpose ----
                zt = [ztp.tile([128, CT], F32, name="zt") for _ in range(2)]
                zsk = []
                for s in range(NSUB):
                    xt = xin.tile([128, D], F32)
                    nc.sync.dma_start(
                        xt[:], x_d[b, t0 + s * 128:t0 + (s + 1) * 128, :])
                    rstd, nmr = ln_scale_bias(xt[:])
                    z = zskip.tile([128, D], F32)
                    nc.scalar.activation(
                        z[:], xt[:], AF.Identity, bias=nmr[:], scale=rstd[:])
                    if g1:
                        nc.vector.tensor_mul(z[:], z[:], gbt["g1"][:])
                        nc.vector.tensor_add(z[:], z[:], gbt["b1"][:])
                    zsk.append(z)
                    zl = zlp.tile([128, D], F32)
                    nc.scalar.activation(zl[:], z[:], AF.Lrelu, alpha=SLOPE)
                    for k in range(2):
                        pt = ptr.tile([128, 128], F32, name="pt", tag="pt")
                        nc.tensor.transpose(
                            pt[:], zl[:, k * 128:(k + 1) * 128], ident[:])
                        nc.scalar.copy(zt[k][:, s * 128:(s + 1) * 128], pt[:])
                # ---- stage 2: B projection -> u in PSUM [n, t] ----
                u = {}
                for comp, bt in ((0, brt), (1, bit)):
                    for p in range(2):
                        ut = pu.tile([128, CT], F32)
                        for k in range(2):
                            nc.tensor.matmul(
                                ut[:], bt[k][:, p * 128:(p + 1) * 128], zt[k][:],
                                start=(k == 0), stop=(k == 1))
                        u[(p, comp)] = ut
                # ---- stage 3: pre-rotation + scans + carry ----
                G = {}
                for p in range(2):
                    if use_mask:
                        d0 = d0p.tile([128, CT], F32)
                        nc.sync.dma_start(
                            d0[:], d0_d[b, p * 128:(p + 1) * 128, t0:t0 + CT])
                        d0ap = d0[:]
                    else:
                        d0ap = rbc[p][:]
                    t1 = tmp.tile([128, CT], F32)
                    nc.vector.tensor_mul(t1[:], cosj[p][:], u[(p, 0)][:])
                    t2 = tmp.tile([128, CT], F32)
                    nc.vector.tensor_mul(t2[:], sinj[p][:], u[(p, 1)][:])
                    wr = wp.tile([128, CT], F32)
                    nc.vector.tensor_add(wr[:], t1[:], t2[:])
                    t3 = tmp.tile([128, CT], F32)
                    nc.vector.tensor_mul(t3[:], cosj[p][:], u[(p, 1)][:])
                    t4 = tmp.tile([128, CT], F32)
                    nc.vector.tensor_mul(t4[:], sneg[p][:], u[(p, 0)][:])
                    wi = wp.tile([128, CT], F32)
                    nc.vector.tensor_add(wi[:], t3[:], t4[:])
                    gr = gp.tile([128, CT], F32)
                    nc.vector.tensor_tensor_scan(
                        gr[:], d0ap, wr[:], ginit[(p, 0)][:],
                        op0=AO.mult, op1=AO.add)
                    gi_t = gp.tile([128, CT], F32)
                    nc.vector.tensor_tensor_scan(
                        gi_t[:], d0ap, wi[:], ginit[(p, 1)][:],
                        op0=AO.mult, op1=AO.add)
                    G[(p, 0)], G[(p, 1)] = gr, gi_t
                    if c + 1 < NCH:
                        ta = statp.tile([128, 1], F32)
                        nc.vector.tensor_mul(ta[:], ecl[p][:], gr[:, CT - 1:CT])
                        tb = statp.tile([128, 1], F32)
                        nc.vector.tensor_mul(tb[:], esl[p][:], gi_t[:, CT - 1:CT])
                        ngr = gip.tile([128, 1], F32)
                        nc.vector.tensor_sub(ngr[:], ta[:], tb[:])
                        tc_ = statp.tile([128, 1], F32)
                        nc.vector.tensor_mul(tc_[:], ecl[p][:], gi_t[:, CT - 1:CT])
                        td = statp.tile([128, 1], F32)
                        nc.vector.tensor_mul(td[:], esl[p][:], gr[:, CT - 1:CT])
                        ngi = gip.tile([128, 1], F32)
                        nc.vector.tensor_add(ngi[:], tc_[:], td[:])
                        ginit[(p, 0)], ginit[(p, 1)] = ngr, ngi
                # ---- stage 4: post-rotation streams ----
                A = {}
                for p in range(2):
                    for idx, (tab, comp) in enumerate(
                            ((cosj, 0), (sneg, 1), (cneg, 1), (sneg, 0))):
                        at = ap_.tile([128, CT], F32)
                        nc.vector.tensor_mul(at[:], tab[p][:], G[(p, comp)][:])
                        A[(p, idx)] = at
                # ---- stage 5: C/D projection + LN2 + leaky + transpose ----
                y2t = [y2tp.tile([128, CT], F32, name="y2t") for _ in range(2)]
                for s in range(NSUB):
                    sl = slice(s * 128, (s + 1) * 128)
                    pt = py1.tile([128, D], F32)
                    mms = []
                    for idx, wgt in ((0, crt), (1, crt), (2, cit), (3, cit)):
                        for p in range(2):
                            mms.append((A[(p, idx)][:, sl], wgt[p][:]))
                    for k in range(2):
                        mms.append((zt[k][:, sl], dts[k][:]))
                    for i, (lhs, rhs) in enumerate(mms):
                        nc.tensor.matmul(pt[:], lhs, rhs, start=(i == 0),
                                         stop=(i == len(mms) - 1))
                    rstd, nmr = ln_scale_bias(pt[:])
                    yl2 = yl2p.tile([128, D], F32)
                    if g2:
                        nc.scalar.activation(
                            yl2[:], pt[:], AF.Identity, bias=nmr[:], scale=rstd[:])
                        nc.vector.tensor_mul(yl2[:], yl2[:], gbt["g2"][:])
                        nc.vector.tensor_add(yl2[:], yl2[:], gbt["b2"][:])
                        nc.scalar.activation(yl2[:], yl2[:], AF.Lrelu, alpha=SLOPE)
                    else:
                        nc.scalar.activation(
                            yl2[:], pt[:], AF.Lrelu, bias=nmr[:], scale=rstd[:],
                            alpha=SLOPE)
                    for p in range(2):
                        ptt = ptr.tile([128, 128], F32, name="pt", tag="pt")
                        nc.tensor.transpose(
                            ptt[:], yl2[:, p * 128:(p + 1) * 128], ident[:])
                        nc.scalar.copy(y2t[p][:, sl], ptt[:])
                # ---- stage 6: MLP + LN3 + skip + store ----
                for s in range(NSUB):
                    sl = slice(s * 128, (s + 1) * 128)
                    p3 = py3.tile([128, D], F32)
                    for p in range(2):
                        nc.tensor.matmul(p3[:], y2t[p][:, sl], mts[p][:],
                                         start=(p == 0), stop=(p == 1))
                    if use_bias:
                        nc.vector.tensor_add(p3[:], p3[:], gbt["mb"][:])
                    rstd, nmr = ln_scale_bias(p3[:])
                    yo = yop.tile([128, D], F32)
                    nc.scalar.activation(
                        yo[:], p3[:], AF.Identity, bias=nmr[:], scale=rstd[:])
                    if g3:
                        nc.vector.tensor_mul(yo[:], yo[:], gbt["g3"][:])
                        nc.vector.tensor_add(yo[:], yo[:], gbt["b3"][:])
                    nc.vector.tensor_add(yo[:], yo[:], zsk[s][:])
                    nc.sync.dma_start(
                        out_d[b, t0 + s * 128:t0 + (s + 1) * 128, :], yo[:])
    nc.compile()
    return nc


def _prep_host(inputs):
    """Host-side precompute: tables, folded weights, per-core input maps."""
    x = np.asarray(inputs["x"], np.float32)
    done = np.asarray(inputs["done"])
    h0r = np.asarray(inputs["h0_re"], np.float32)
    h0i = np.asarray(inputs["h0_im"], np.float32)
    nu = np.asarray(inputs["nu_log"], np.float64)
    th_log = np.asarray(inputs["theta_log"], np.float64)
    gl = np.asarray(inputs["gamma_log"], np.float64)

    r = np.exp(-np.exp(nu))                     # |lambda|, [N]
    theta = np.exp(th_log)                      # [N]
    gamma = np.exp(gl)

    j = np.arange(CT, dtype=np.float64)
    ang = theta[:, None] * j[None, :]           # [N, CT]
    cosj = np.cos(ang).astype(np.float32)
    sinj = np.sin(ang).astype(np.float32)
    cneg = (-np.cos(ang)).astype(np.float32)
    sneg = (-np.sin(ang)).astype(np.float32)
    rbc = np.repeat(r.astype(np.float32)[:, None], CT, axis=1)
    angL = theta * CT
    ecl = np.cos(angL).astype(np.float32)[:, None]
    esl = np.sin(angL).astype(np.float32)[:, None]

    # q0 = e^{i theta} * h0  per (b, n)
    c1, s1 = np.cos(theta), np.sin(theta)
    q0r = (c1[None, :] * h0r - s1[None, :] * h0i).astype(np.float32)
    q0i = (c1[None, :] * h0i + s1[None, :] * h0r).astype(np.float32)

    brt = np.ascontiguousarray(
        (np.asarray(inputs["B_re"], np.float64) * gamma[:, None]).T
    ).astype(np.float32)
    bit = np.ascontiguousarray(
        (np.asarray(inputs["B_im"], np.float64) * gamma[:, None]).T
    ).astype(np.float32)
    crt = np.ascontiguousarray(np.asarray(inputs["C_re"], np.float32).T)
    cit = np.ascontiguousarray(np.asarray(inputs["C_im"], np.float32).T)
    dt = np.ascontiguousarray(np.asarray(inputs["D_mat"], np.float32).T)
    mt = np.ascontiguousarray(np.asarray(inputs["mlp_w"], np.float32).T)

    g1v = np.asarray(inputs["ln1_g"], np.float32)
    b1v = np.asarray(inputs["ln1_b"], np.float32)
    g2v = np.asarray(inputs["ln2_g"], np.float32)
    b2v = np.asarray(inputs["ln2_b"], np.float32)
    g3v = np.asarray(inputs["ln3_g"], np.float32)
    b3v = np.asarray(inputs["ln3_b"], np.float32)
    mbv = np.asarray(inputs["mlp_b"], np.float32)

    g1 = not (np.all(g1v == 1) and np.all(b1v == 0))
    g2 = not (np.all(g2v == 1) and np.all(b2v == 0))
    g3 = not (np.all(g3v == 1) and np.all(b3v == 0))
    use_bias = bool(np.any(mbv != 0))
    use_mask = bool(np.any(done))
    flags = (g1, g2, g3, use_bias, use_mask)

    shared = dict(cosj=cosj, sinj=sinj, cneg=cneg, sneg=sneg, rbc=rbc,
                  ecl=ecl, esl=esl, BrT=brt, BiT=bit, CrT=crt, CiT=cit,
                  DT=dt, MT=mt)

    def bc(v):
        return np.ascontiguousarray(np.broadcast_to(v[None, :], (128, D))
                                    ).astype(np.float32)
    if g1:
        shared["g1bc"], shared["b1bc"] = bc(g1v), bc(b1v)
    if g2:
        shared["g2bc"], shared["b2bc"] = bc(g2v), bc(b2v)
    if g3:
        shared["g3bc"], shared["b3bc"] = bc(g3v), bc(b3v)
    if use_bias:
        shared["mbbc"] = bc(mbv)

    in_maps = []
    for core in range(NCORES):
        sl = slice(core * BS, (core + 1) * BS)
        m = dict(shared)
        m["x"] = np.ascontiguousarray(x[sl])
        m["q0r"] = np.ascontiguousarray(q0r[sl])
        m["q0i"] = np.ascontiguousarray(q0i[sl])
        if use_mask:
            mask = 1.0 - done[sl].astype(np.float32)       # [BS, T]
            d0 = (rbc[None, :, 0:1] * mask[:, None, :])    # [BS, N, T]
            m["d0tab"] = np.ascontiguousarray(d0.astype(np.float32))
        in_maps.append(m)
    return flags, in_maps


def _get_program(flags):
    if flags not in _PROG_CACHE:
        _PROG_CACHE[flags] = _build_program(flags)
    return _PROG_CACHE[flags]


def run(inputs, trace=False, **kw):
    flags, in_maps = _prep_host(inputs)
    nc = _get_program(flags)
    res = run_bass_kernel_spmd(nc, in_maps, list(range(NCORES)),
                               trace=trace, **kw)
    out = np.concatenate([res.results[i]["out"] for i in range(NCORES)], axis=0)
    return out, res


def kernel(**inputs):
    out, _ = run(inputs, trace=False)
    return out

